# revision 44
# baseline (speedup 1.0000x reference)
"""Trainium2 Bass kernel for nn_CNLinkPredictor (gnn_message_passing).

Strategy: data-parallel over target edges T (8192) across 8 NeuronCores
(1024 edges/core). Per core, the per-edge CN-token transformer is computed
in tiles of 4 edges = 128 tokens (tokens on SBUF partitions).

Layout/algorithm notes:
 - pf = [xw|xi|xj|xi*xj] @ tok_w.T is split: the xw part is a per-token
   matmul; the (xi,xj,xi*xj) part depends only on the edge -> computed once
   per edge ("EC") and broadcast to the edge's 32 tokens via a rank-4 matmul.
 - LayerNorm affine transforms (gamma/beta) are folded into the following
   matmul weights on the host; device LN computes only (x-mu)*rsqrt(var+eps).
 - qT/kT are produced channel-major with heads padded to 32-partition slots
   so score matmuls can use 32-aligned lhsT partition slices.
 - scores land in PSUM [128 ktok, 8 heads x 128 qtok]; masking (block-diag
   cross-edge + key-padding) is folded into the exp's per-partition bias
   (4 activation calls, one per edge in the tile).
 - softmax denominators ride along as a 17th column of V ("aug-V"); ctx
   matmuls produce [17, 128] blocks (16 ctx channels + denominator row) in
   32-partition head slots; normalization = stream_shuffle + divide.
 - v bias, out_proj bias, ff biases etc. are folded on host where linear.
"""

import sys
import threading

sys.path.insert(0, "/opt/trn_rl_repo")

import numpy as np
import ml_dtypes

import concourse.bass as bass
import concourse.bacc as bacc
import concourse.mybir as mybir
from concourse.tile import TileContext
from concourse.masks import make_identity
from concourse.bass_utils import run_bass_kernel_spmd

F32 = mybir.dt.float32
BF = mybir.dt.bfloat16
BF_NP = ml_dtypes.bfloat16
I32 = mybir.dt.int32
AF = mybir.ActivationFunctionType
ALU = mybir.AluOpType

N, C, H, O = 100000, 128, 256, 1
T, K = 8192, 32
NHEAD, DH, FF = 8, 16, 512
NCORES = 8
TC = T // NCORES          # 1024 edges per core
NT = TC * K // 128        # 256 main tiles (4 edges / 128 tokens each)
NE = TC // 128            # 8 edge tiles (phase A)
NEG = -30.0
EPS_DENOM = 1e-30


def _build_nc(nt=NT, phases="abc", bsteps=99):
    assert nt % 32 == 0
    tcn = 4 * nt           # edges covered by this build
    ne = tcn // 128        # phase-A tiles
    nc = bacc.Bacc("TRN2", target_bir_lowering=False)

    dt = {}

    def din(name, shape, dtype=F32):
        dt[name] = nc.dram_tensor(name, shape, dtype, kind="ExternalInput")
        return dt[name]

    # data
    din("x", [N, C])
    din("idx_cn", [128, nt], I32)
    din("idx_t0", [128, ne], I32)
    din("idx_t1", [128, ne], I32)
    din("vselfneg", [128, nt])
    din("ebd", [128, 4 * nt], BF)
    din("ind", [1, 4 * nt], BF)
    # weights / constants
    for nm in ["w0xT", "a1", "a2", "a3", "wq_l", "wk_even", "wk_odd",
               "woutA2", "woutB2"]:
        din(nm, [128, 128], BF)
    din("wv_aug2", [128, 137], BF)
    din("sel_all", [128, 32 * 128], BF)
    din("sel17", [1, 137], BF)
    din("bff1q", [4, 128], BF)
    din("ind4x", [4, 512], BF)
    din("ones_col", [128, 1], BF)
    for nm in ["wff1_0", "wff1_1", "wff1_2", "wff1_3",
               "wff2_0", "wff2_1", "wff2_2", "wff2_3"]:
        din(nm, [128, 128], BF)
    for nm in ["wx1_0", "wx1_1", "wxj1_0", "wxj1_1"]:
        din(nm, [128, 128], BF)
    for ic in range(2):
        for oc in range(2):
            din(f"wx2_{ic}{oc}", [128, 128], BF)
            din(f"wx3_{ic}{oc}", [128, 128], BF)
            din(f"wxj2_{ic}{oc}", [128, 128], BF)
            din(f"wl1_{ic}{oc}", [128, 128], BF)
    din("wl2_0", [128, 1], BF)
    din("wl2_1", [128, 1], BF)
    din("sel4", [4, 128], BF)
    din("ones1", [1, 128], BF)
    din("tokb_row", [1, 128], BF)
    din("bff2_row", [1, 128], BF)
    for nm in ["beta_col", "eps_col", "epsd_col"]:
        din(nm, [128, 1])
    for nm in ["bx1_0", "bx1_1", "bx2_0", "bx2_1", "bx3_0", "bx3_1",
               "bxj1_0", "bxj1_1", "bxj2_0", "bxj2_1", "bl1_0", "bl1_1"]:
        din(nm, [128, 1])
    din("bl2", [1, 1])

    out_dram = nc.dram_tensor("out", [1, 4 * nt], F32, kind="ExternalOutput")

    with TileContext(nc) as tc:
        with (
            tc.tile_pool(name="cpool", bufs=1) as cp,
            tc.tile_pool(name="wpool", bufs=6) as wp,
            tc.tile_pool(name="mlppool", bufs=1) as mp,
            tc.tile_pool(name="carry", bufs=24) as cr,
            tc.tile_pool(name="ps", bufs=2, space="PSUM") as ps,
            tc.tile_pool(name="psattn", bufs=1, space="PSUM") as pa,
            tc.tile_pool(name="psff", bufs=1, space="PSUM") as pf,
            tc.tile_pool(name="pspool", bufs=1, space="PSUM") as ps2,
        ):
            # ---- load constants to SBUF --------------------------------
            cs = {}
            for nm, t in dt.items():
                if nm == "x":
                    continue
                tile = cp.tile(list(t.shape), t.dtype, tag=f"c_{nm}")
                nc.sync.dma_start(tile[:], t[:])
                cs[nm] = tile

            ident = cp.tile([128, 128], F32, tag="ident")
            make_identity(nc, ident[:])
            identB = cp.tile([128, 128], BF, tag="identB")
            make_identity(nc, identB[:])

            xijT_all = cp.tile([128, tcn], BF, tag="xijT_all")
            xcnT_all = cp.tile([128, tcn], BF, tag="xcnT_all")
            ec_sb = cp.tile([128, ne * 128], BF, tag="ec_sb")

            def transpose_to(dst_ap, src_ap, eng="act", idt=None):
                tp = ps.tile([128, 128], F32 if idt is None else BF,
                             tag="p128")
                nc.tensor.transpose(tp[:], src_ap,
                                    (ident if idt is None else idt)[:])
                if eng == "act":
                    nc.scalar.copy(dst_ap, tp[:])
                else:
                    nc.vector.tensor_copy(out=dst_ap, in_=tp[:])

            # ---- PHASE A: per-edge features EC + xijT ------------------
            # all 16 gathers issued upfront so the SWDGE queue pipelines
            # them and the per-j compute (and phase B group 0) overlaps.
            na = ne if "a" in phases else 0
            xi_all = cp.tile([128, max(na, 1) * C], F32, tag="xi_all")
            xj_all = cp.tile([128, max(na, 1) * C], F32, tag="xj_all")
            for j in range(na):
                nc.gpsimd.indirect_dma_start(
                    out=xi_all[:, C * j:C * (j + 1)], out_offset=None,
                    in_=dt["x"][:],
                    in_offset=bass.IndirectOffsetOnAxis(
                        ap=cs["idx_t0"][:, j:j + 1], axis=0))
                nc.gpsimd.indirect_dma_start(
                    out=xj_all[:, C * j:C * (j + 1)], out_offset=None,
                    in_=dt["x"][:],
                    in_offset=bass.IndirectOffsetOnAxis(
                        ap=cs["idx_t1"][:, j:j + 1], axis=0))
            for j in range(na):
                xi = xi_all[:, C * j:C * (j + 1)]
                xj = xj_all[:, C * j:C * (j + 1)]
                xij = wp.tile([128, C], F32, tag="xij")
                nc.vector.tensor_tensor(out=xij[:], in0=xi, in1=xj,
                                        op=ALU.mult)
                xiT = wp.tile([128, 128], BF, tag="xiT")
                xjT = wp.tile([128, 128], BF, tag="xjT")
                transpose_to(xiT[:], xi)
                transpose_to(xjT[:], xj)
                transpose_to(xijT_all[:, 128 * j:128 * (j + 1)], xij[:])

                ecp = ps.tile([128, 128], F32, tag="p128")
                nc.tensor.matmul(ecp[:], lhsT=xiT[:], rhs=cs["a1"][:],
                                 start=True, stop=False)
                nc.tensor.matmul(ecp[:], lhsT=xjT[:], rhs=cs["a2"][:],
                                 start=False, stop=False)
                nc.tensor.matmul(ecp[:], lhsT=xijT_all[:, 128 * j:128 * (j + 1)],
                                 rhs=cs["a3"][:], start=False, stop=False)
                nc.tensor.matmul(ecp[:], lhsT=cs["ones1"][:],
                                 rhs=cs["tokb_row"][:], start=False, stop=True)
                nc.scalar.copy(ec_sb[:, 128 * j:128 * (j + 1)], ecp[:])

            # ---- PHASE B: grouped, staged over 128-token tiles ---------
            # Staged per group of GRP tiles so ACT table funcs (sqrt/exp/
            # gelu) batch together. Within a group, 4-tile subgroups share
            # batched [128, 512] q/k matmuls.
            SHUF16 = [16] * 32
            GRP = 32
            SG = 4
            nb = nt if "b" in phases else 0

            def s1a(m, mvg, i, xw):
                """tok (relu'd) + LN1 stats into mvg col i; xw pre-gathered."""
                u = m % 32
                jc = (4 * m) // 128
                xwT = wp.tile([128, 128], BF, tag="xwT", name="xwT")
                transpose_to(xwT[:], xw, eng="dve", idt=identB)
                tokp = ps.tile([128, 128], F32, tag="p128", name="tokp")
                nc.tensor.matmul(tokp[:], lhsT=xwT[:], rhs=cs["w0xT"][:],
                                 start=True, stop=False)
                nc.tensor.matmul(tokp[:], lhsT=cs["sel_all"][:, 128 * u:
                                                             128 * (u + 1)],
                                 rhs=ec_sb[:, 128 * jc:128 * (jc + 1)],
                                 start=False, stop=True)
                tok = cr.tile([128, 128], BF, tag="tok", name="tok",
                              bufs=2 * GRP + 16)
                nc.scalar.activation(tok[:], tokp[:], AF.Relu)
                st = wp.tile([128, 6], F32, tag="ln_st", name="st")
                nc.vector.bn_stats(st[:], tok[:])
                nc.vector.bn_aggr(mvg[:, 2 * i:2 * i + 2], st[:])
                return tok

            def group_rstd(mvg, lo, n):
                """[128, 2*GRP] rstd tile from interleaved (mean, var) pairs
                in mvg cols [2*lo, 2*(lo+n)); one sqrt + one recip."""
                sg = cr.tile([128, 2 * GRP], F32, tag="sg", name="sg")
                mview = mvg[:].rearrange("p (i t) -> p i t", t=2)
                sview = sg[:, lo:lo + n].rearrange("p (i t) -> p i t", t=1)
                nc.scalar.activation(sview, mview[:, lo:lo + n, 1:2],
                                     AF.Sqrt, bias=cs["eps_col"][:, 0:1])
                rg = cr.tile([128, 2 * GRP], F32, tag="rg", name="rg")
                nc.vector.reciprocal(rg[:, lo:lo + n], sg[:, lo:lo + n])
                return rg

            def ln_apply(x, mvg, rg, i, tag):
                z = wp.tile([128, 128], BF, tag="z_" + tag, name="z")
                nc.vector.tensor_scalar(out=z[:], in0=x[:],
                                        scalar1=mvg[:, 2 * i:2 * i + 1],
                                        scalar2=rg[:, i:i + 1],
                                        op0=ALU.subtract, op1=ALU.mult)
                return z

            SW = 128 * SG

            def sub_qkv(z1T4):
                """Batched q / k_even / k_odd over the subgroup's tokens."""
                qkp = pa.tile([128, 2048], F32, tag="attn4", name="qkp")
                nc.tensor.matmul(qkp[:, 0:SW], lhsT=cs["wq_l"][:],
                                 rhs=z1T4[:], start=True, stop=True)
                nc.tensor.matmul(qkp[:, 512:512 + SW], lhsT=cs["wk_even"][:],
                                 rhs=z1T4[:], start=True, stop=True)
                nc.tensor.matmul(qkp[:, 1024:1024 + SW], lhsT=cs["wk_odd"][:],
                                 rhs=z1T4[:], start=True, stop=True)
                qT4 = wp.tile([128, SW], BF, tag="qT4", name="qT4")
                nc.scalar.copy(qT4[:], qkp[:, 0:SW])
                kTe4 = wp.tile([128, SW], BF, tag="kTe4", name="kTe4")
                nc.vector.tensor_copy(out=kTe4[:], in_=qkp[:, 512:512 + SW])
                kTo4 = wp.tile([128, SW], BF, tag="kTo4", name="kTo4")
                nc.scalar.copy(kTo4[:], qkp[:, 1024:1024 + SW])
                return qT4, kTe4, kTo4

            def s1b(m, t, tok, qT4, kTe4, kTo4, z1T):
                """Attention for one tile: v, scores, exp, ctx, out-proj."""
                # scores: 8 head matmuls, contraction 32 (head pair slots),
                # head h -> bank h//2, col 128*(h%2). The v matmul (full
                # array, any bank legal) rides in bank-0 spare cols.
                sp = pa.tile([128, 2048], F32, tag="attn4", name="sp")
                nc.tensor.matmul(sp[:, 256:393], lhsT=z1T[:],
                                 rhs=cs["wv_aug2"][:], start=True, stop=False)
                nc.tensor.matmul(sp[:, 256:393], lhsT=cs["ones1"][:],
                                 rhs=cs["sel17"][:], start=False, stop=True)
                vaug = wp.tile([128, 8 * 17], BF, tag="vaug", name="vaug",
                               bufs=5)
                nc.vector.tensor_copy(out=vaug[:], in_=sp[:, 256:392])
                ebias = wp.tile([128, 1], F32, tag="ebias", name="ebias",
                                bufs=5)
                nc.vector.tensor_tensor(
                    out=ebias[:], in0=sp[:, 392:393],
                    in1=cs["vselfneg"][:, m:m + 1], op=ALU.add)
                for h in range(8):
                    s = h // 2
                    src = kTe4 if h % 2 == 0 else kTo4
                    c0 = 512 * s + 32 * (h % 2)
                    for e in range(4):
                        nc.tensor.matmul(
                            sp[32 * e:32 * e + 32, c0:c0 + 32],
                            lhsT=src[32 * s:32 * s + 32,
                                     128 * t + 32 * e:128 * t + 32 * e + 32],
                            rhs=qT4[32 * s:32 * s + 32,
                                    128 * t + 32 * e:128 * t + 32 * e + 32],
                            start=True, stop=True,
                            tile_position=(32 * s, 32 * e))
                E = wp.tile([128, 256], BF, tag="E", name="E", bufs=5)
                sview = sp[:].rearrange("p (s r) -> p s r", s=4)[:, :, 0:64]
                eview = E[:].rearrange("p (s r) -> p s r", s=4)
                nc.scalar.activation(eview, sview, AF.Exp,
                                     bias=ebias[:, 0:1])

                # ctx: 32 small matmuls (e, h), 16-way tile_position packing
                cxp = pa.tile([128, 2048], F32, tag="attn4", name="cxp")
                for hh in range(2):
                    for h4 in range(4):
                        h = 4 * hh + h4
                        for e in range(4):
                            nc.tensor.matmul(
                                cxp[32 * h4:32 * h4 + 17,
                                    512 * e + 32 * hh:512 * e + 32 * hh + 32],
                                lhsT=vaug[32 * e:32 * e + 32,
                                          17 * h:17 * h + 17],
                                rhs=E[32 * e:32 * e + 32,
                                      32 * h:32 * h + 32],
                                start=True, stop=True,
                                tile_position=(32 * e, 32 * h4))
                # evacuate ctx to SBUF promptly (releases the attn4 buffer
                # so the next tile's scores can start); the norm chain then
                # runs off the critical path. Col-permute (e, hh, q) ->
                # (hh, e, q) so the out-proj lhsT is a contiguous 128-slice.
                # Padding keys contribute exp(-30) ~ 1e-13, so denominators
                # are always > 0 (no eps needed).
                cx = wp.tile([128, 256], F32, tag="cx", name="cx", bufs=5)
                cview = (cxp[:].rearrange("p (e r) -> p e r", e=4)
                         [:, :, 0:64].rearrange("p e (h q) -> p e h q", q=32))
                nc.scalar.copy(
                    cx[:].rearrange("p (h e q) -> p e h q", e=4, q=32), cview)
                rt = wp.tile([128, 256], F32, tag="rt", name="rt")
                nc.vector.stream_shuffle(rt[:], cx[:], SHUF16)
                rtr = wp.tile([128, 256], F32, tag="rtr", name="rtr")
                nc.vector.reciprocal_approx_fast(rtr[:], rt[:])
                cn = wp.tile([128, 256], BF, tag="cn", name="cn", bufs=5)
                nc.vector.tensor_tensor(out=cn[:], in0=cx[:], in1=rtr[:],
                                        op=ALU.mult)

                # out-proj: lhsT = cn cols regrouped per head-half; the
                # normalized denominator row (==1) carries the output bias
                # through woutA2's slot-0 row 16. up lives in the p128 pool
                # so the attn4 buffer is released after the cx evacuation.
                upt = ps.tile([128, 128], F32, tag="p128", name="upt")
                up = upt[:]
                nc.tensor.matmul(up, lhsT=cn[:, 0:128],
                                 rhs=cs["woutA2"][:], start=True, stop=False)
                nc.tensor.matmul(up, lhsT=cn[:, 128:256],
                                 rhs=cs["woutB2"][:], start=False, stop=True)
                tok2 = cr.tile([128, 128], BF, tag="tok2", name="tok2",
                               bufs=GRP + 16)
                nc.vector.tensor_tensor(out=tok2[:], in0=tok[:], in1=up,
                                        op=ALU.add)
                return tok2

            def s1b_stats(tok2, mvg2, i):
                st2 = wp.tile([128, 6], F32, tag="ln_st2", name="st2")
                nc.vector.bn_stats(st2[:], tok2[:])
                nc.vector.bn_aggr(mvg2[:, 2 * i:2 * i + 2], st2[:])

            pool_hold = {}

            def s2(m, tok2, mvg2, rg2, i):
                """LN2 apply + ff + residual + pool."""
                z2 = ln_apply(tok2, mvg2, rg2, i, "2")
                z2T = wp.tile([128, 128], BF, tag="z2T", name="z2T")
                transpose_to(z2T[:], z2[:], eng="dve", idt=identB)
                gT = wp.tile([128, 512], BF, tag="gT", name="gT")
                fp = pf.tile([128, 512], F32, tag="big", name="fp")
                nc.tensor.matmul(fp[:], lhsT=cs["bff1q"][:],
                                 rhs=cs["ind4x"][:], start=True, stop=False)
                for c4 in range(4):
                    nc.tensor.matmul(fp[:, 128 * c4:128 * (c4 + 1)],
                                     lhsT=cs[f"wff1_{c4}"][:],
                                     rhs=z2T[:], start=False,
                                     stop=(c4 == 3), skip_group_check=True)
                nc.scalar.activation(gT[:], fp[:], AF.Gelu)
                f2p = ps.tile([128, 128], F32, tag="p128", name="f2p")
                for c4 in range(4):
                    nc.tensor.matmul(f2p[:],
                                     lhsT=gT[:, 128 * c4:128 * (c4 + 1)],
                                     rhs=cs[f"wff2_{c4}"][:],
                                     start=(c4 == 0), stop=(c4 == 3))
                tok3 = wp.tile([128, 128], BF, tag="tok3", name="tok3")
                nc.vector.tensor_tensor(out=tok3[:], in0=tok2[:], in1=f2p[:],
                                        op=ALU.add)
                # transposed pool: xcnT[:, 4m:4m+4] = tok3.T @ ebd + bff2*ind
                # accumulated into one long-lived PSUM bank; evacuated to
                # SBUF once per 128 tiles.
                if pool_hold.get("m0") is None:
                    pool_hold["tile"] = ps2.tile([128, 512], F32, tag="pool",
                                                 name="ppool")
                    pool_hold["m0"] = m
                pp = pool_hold["tile"]
                o = 4 * (m - pool_hold["m0"])
                nc.tensor.matmul(pp[:, o:o + 4], lhsT=tok3[:],
                                 rhs=cs["ebd"][:, 4 * m:4 * m + 4],
                                 start=True, stop=False)
                nc.tensor.matmul(pp[:, o:o + 4], lhsT=cs["bff2_row"][0:1, :],
                                 rhs=cs["ind"][0:1, 4 * m:4 * m + 4],
                                 start=False, stop=True)
                if o == 508 or m == nb - 1:
                    m0 = pool_hold["m0"]
                    nc.vector.tensor_copy(
                        out=xcnT_all[:, 4 * m0:4 * m + 4],
                        in_=pp[:, 0:o + 4])
                    pool_hold["m0"] = None

            def make_s1a_units(gms2, mvg1n, toks2):
                """One emission thunk per tile of the NEXT group; the first
                of each 4-subgroup also emits that subgroup's gathers."""
                units = []
                for i0 in range(0, len(gms2), SG):
                    sub = gms2[i0:i0 + SG]
                    holder = {}
                    for tt, m in enumerate(sub):
                        def unit(m=m, tt=tt, i0=i0, sub=sub, holder=holder):
                            if tt == 0:
                                xw4 = wp.tile([128, SG * C], BF, tag="xw4",
                                              name="xw4", bufs=4)
                                for uu, mu in enumerate(sub):
                                    nc.gpsimd.indirect_dma_start(
                                        out=xw4[:, C * uu:C * (uu + 1)],
                                        out_offset=None, in_=dt["x"][:],
                                        in_offset=bass.IndirectOffsetOnAxis(
                                            ap=cs["idx_cn"][:, mu:mu + 1],
                                            axis=0))
                                holder["xw4"] = xw4
                            toks2[m] = s1a(m, mvg1n, GRP + i0 + tt,
                                           holder["xw4"][:,
                                                         C * tt:C * (tt + 1)])
                        units.append(unit)
                return units

            groups = [list(range(g0, min(g0 + GRP, nb)))
                      for g0 in range(0, nb, GRP)]
            pend = None          # (gms, tok2s, mvgc) of the previous group
            for gi, gms in enumerate(groups):
                ng = len(gms)
                mvg_in = (pend[2] if pend else
                          cr.tile([128, 4 * GRP], F32, tag="mvgc",
                                  name="mvgc", bufs=2))
                toks = {}
                for u in make_s1a_units(gms, mvg_in, toks):
                    u()
                # rg2p early (unblocks prev group s2 immediately); rg1 after
                # this group's s1a stats land.
                rg2p = group_rstd(pend[2], 0, len(pend[0])) if pend else None
                rgc = group_rstd(mvg_in, GRP, ng)
                mvgc = mvg_in
                if pend:
                    for ii, mm in enumerate(pend[0]):
                        s2(mm, pend[1][mm], pend[2], rg2p, ii)
                mvg2 = cr.tile([128, 4 * GRP], F32, tag="mvgc", name="mvgc",
                               bufs=2)
                tok2s = {}
                for i0 in range(0, ng, SG):
                    sub = gms[i0:i0 + SG]
                    z1T4 = wp.tile([128, 128 * SG], BF, tag="z1T4",
                                   name="z1T4", bufs=2)
                    for tt, m in enumerate(sub):
                        z1 = ln_apply(toks[m], mvgc, rgc, GRP + i0 + tt, "1")
                        transpose_to(z1T4[:, 128 * tt:128 * (tt + 1)], z1[:],
                                     idt=identB)
                    qT4, kTe4, kTo4 = sub_qkv(z1T4)
                    for tt, m in enumerate(sub):
                        tok2s[m] = s1b(m, tt, toks[m], qT4, kTe4, kTo4,
                                       z1T4[:, 128 * tt:128 * (tt + 1)])
                # LN2 stats emitted after the attention sweep so they queue
                # behind (not between) the per-tile norm-chain DVE ops.
                for ii, m in enumerate(gms):
                    s1b_stats(tok2s[m], mvg2, ii)
                pend = (gms, tok2s, mvg2)
            if pend:
                rg2p = group_rstd(pend[2], 0, len(pend[0]))
                for i, m in enumerate(pend[0]):
                    s2(m, pend[1][m], pend[2], rg2p, i)

            # ---- PHASE C ------------------------------------
            def _phase_c(lo, w):
                # ---- PHASE C: edge MLPs (edges [lo, lo+w)) -----------------
                xcnT = xcnT_all[:, lo:lo + w]

                def dense(rhs_tile, win, bin_, act, n_ic, out_tag):
                    """out[oc-chunk][128, w] = act(W @ rhs + b)."""
                    outs = []
                    for oc in range(2):
                        o = mp.tile([128, w], BF, tag=f"{out_tag}{oc}",
                                    name=out_tag)
                        for nh in range(max(1, w // 512)):
                            cw = min(512, w)
                            p5 = pf.tile([128, 512], F32, tag="big")
                            for ic in range(n_ic):
                                wt = cs[win(ic, oc)]
                                r = (rhs_tile if n_ic == 1 else rhs_tile[ic])
                                nc.tensor.matmul(
                                    p5[:, :cw], lhsT=wt[:],
                                    rhs=r[:, cw * nh:cw * (nh + 1)],
                                    start=(ic == 0), stop=(ic == n_ic - 1))
                            nc.scalar.activation(
                                o[:, cw * nh:cw * (nh + 1)], p5[:, :cw], act,
                                bias=cs[bin_(oc)][:, 0:1])
                        outs.append(o)
                    return outs

                h1 = dense(xcnT, lambda ic, oc: f"wx1_{oc}",
                           lambda oc: f"bx1_{oc}", AF.Relu, 1, "h1_")
                h2 = dense(h1, lambda ic, oc: f"wx2_{ic}{oc}",
                           lambda oc: f"bx2_{oc}", AF.Relu, 2, "h2_")
                h3 = dense(h2, lambda ic, oc: f"wx3_{ic}{oc}",
                           lambda oc: f"bx3_{oc}", AF.Identity, 2, "h3_")
                j1 = dense(xijT_all[:, lo:lo + w], lambda ic, oc: f"wxj1_{oc}",
                           lambda oc: f"bxj1_{oc}", AF.Relu, 1, "j1_")
                j2 = dense(j1, lambda ic, oc: f"wxj2_{ic}{oc}",
                           lambda oc: f"bxj2_{oc}", AF.Identity, 2, "j2_")
                zi = []
                for oc in range(2):
                    z = mp.tile([128, w], BF, tag=f"zi{oc}", name="zi")
                    nc.vector.scalar_tensor_tensor(
                        out=z[:], in0=h3[oc][:], scalar=cs["beta_col"][:, 0:1],
                        in1=j2[oc][:], op0=ALU.mult, op1=ALU.add)
                    zi.append(z)
                zz = dense(zi, lambda ic, oc: f"wl1_{ic}{oc}",
                           lambda oc: f"bl1_{oc}", AF.Relu, 2, "zz")

                osb = mp.tile([1, w], F32, tag="osb", name="osb")
                cw = min(512, w)
                for nh in range(max(1, w // 512)):
                    fo = ps.tile([1, 512], F32, tag="p128")
                    nc.tensor.matmul(fo[:, :cw], lhsT=cs["wl2_0"][:],
                                     rhs=zz[0][:, cw * nh:cw * (nh + 1)],
                                     start=True, stop=False)
                    nc.tensor.matmul(fo[:, :cw], lhsT=cs["wl2_1"][:],
                                     rhs=zz[1][:, cw * nh:cw * (nh + 1)],
                                     start=False, stop=True)
                    nc.scalar.activation(osb[0:1, cw * nh:cw * (nh + 1)],
                                         fo[:, :cw],
                                         AF.Identity, bias=cs["bl2"][0:1, 0:1])
                nc.sync.dma_start(out_dram[0:1, lo:lo + w], osb[:])

            if "c" in phases:
                for _lo in range(0, tcn, 512):
                    _phase_c(_lo, min(512, tcn - _lo))
            else:
                dumm = mp.tile([1, tcn], F32, tag="dumm")
                nc.vector.memset(dumm[:], 0.0)
                nc.sync.dma_start(out_dram[:], dumm[:])

    nc.finalize()
    return nc


def _ln_stats(nc, wp, x, z_out, eps_col):
    """z = (x - mean(x)) / sqrt(var(x) + 1e-5) along the free dim."""
    st = wp.tile([128, 6], F32, tag="ln_st")
    nc.vector.bn_stats(st[:], x[:])
    mv = wp.tile([128, 2], F32, tag="ln_mv")
    nc.vector.bn_aggr(mv[:], st[:])
    std = wp.tile([128, 1], F32, tag="ln_std")
    nc.scalar.activation(std[:], mv[:, 1:2], AF.Sqrt, bias=eps_col[:, 0:1])
    rstd = wp.tile([128, 1], F32, tag="ln_rstd")
    nc.vector.reciprocal(rstd[:], std[:])
    nc.vector.tensor_scalar(out=z_out[:], in0=x[:], scalar1=mv[:, 0:1],
                            scalar2=rstd[:, 0:1], op0=ALU.subtract,
                            op1=ALU.mult)


# ---------------------------------------------------------------- host side

def _slot_pad_w(Weff, beff, heads):
    """[128c, 128slot] lhsT with 4 heads in 32-slots (16 data + 16 zero)."""
    w = np.zeros((128, 128), np.float32)
    b = np.zeros((128, 1), np.float32)
    for i, h in enumerate(heads):
        w[:, 32 * i:32 * i + 16] = Weff[16 * h:16 * h + 16, :].T
        b[32 * i:32 * i + 16, 0] = beff[16 * h:16 * h + 16]
    return w, b


def _prep_shared(inp):
    f = lambda k: np.asarray(inp[k], np.float32)
    tok_w, tok_b = f("tok_w"), f("tok_b")
    g1, b1 = f("ln1_g"), f("ln1_b")
    qkv_w, qkv_b = f("qkv_w"), f("qkv_b")
    out_w, out_b = f("out_w"), f("out_b")
    g2, b2 = f("ln2_g"), f("ln2_b")
    ff1_w, ff1_b = f("ff1_w"), f("ff1_b")
    ff2_w, ff2_b = f("ff2_w"), f("ff2_b")

    d = {}
    d["w0xT"] = tok_w[:, :C].T.copy()
    d["a1"] = tok_w[:, C:2 * C].T.copy()
    d["a2"] = tok_w[:, 2 * C:3 * C].T.copy()
    d["a3"] = tok_w[:, 3 * C:4 * C].T.copy()
    d["tokb_row"] = tok_b[None, :].copy()

    sc = 1.0 / np.sqrt(DH)
    Wq, Wk, Wv = qkv_w[:C], qkv_w[C:2 * C], qkv_w[2 * C:3 * C]
    bq, bk, bv = qkv_b[:C], qkv_b[C:2 * C], qkv_b[2 * C:3 * C]
    Wq_e = Wq * g1[None, :] * sc
    bq_e = (Wq @ b1) * sc + bq * sc
    Wk_e = Wk * g1[None, :]
    Wv_e = Wv * g1[None, :]
    bv_e = Wv @ b1 + bv
    # q: plain channel-major; k: two copies with alternate 16-chan head
    # blocks zeroed (scores contract 32-part slots holding a head pair).
    d["wq_l"] = Wq_e.T.copy()
    ke = Wk_e.T.copy()
    ko = Wk_e.T.copy()
    for h in range(8):
        if h % 2 == 1:
            ke[:, 16 * h:16 * h + 16] = 0.0
        else:
            ko[:, 16 * h:16 * h + 16] = 0.0
    d["wk_even"] = ke
    d["wk_odd"] = ko
    # q/k additive biases: per-qtok terms cancel in softmax; the per-ktok
    # term c_k = bq_e . k(token) rides as v output col 136 -> exp bias.
    w_ck = Wk_e.T @ bq_e                      # [128 in-c]
    wv2 = np.zeros((128, 137), np.float32)
    for h in range(8):
        wv2[:, 17 * h:17 * h + 16] = Wv_e.T[:, 16 * h:16 * h + 16]
    wv2[:, 136] = w_ck
    d["wv_aug2"] = wv2
    # out-proj: slot s' rows 32s'+c = out_w col for head 4*hh+s'; the
    # normalized denominator row (==1) in slot 0 carries the bias.
    outb_eff = out_b + out_w @ bv_e
    for nm, hh in (("woutA2", 0), ("woutB2", 1)):
        w = np.zeros((128, 128), np.float32)
        for s4 in range(4):
            h = 4 * hh + s4
            w[32 * s4:32 * s4 + 16, :] = out_w[:, 16 * h:16 * h + 16].T
        if hh == 0:
            w[16, :] += outb_eff
        d[nm] = w

    bff1q = np.zeros((4, 128), np.float32)
    ind4x = np.zeros((4, 512), np.float32)
    for c4 in range(4):
        sl = slice(128 * c4, 128 * (c4 + 1))
        d[f"wff1_{c4}"] = (ff1_w[sl, :] * g2[None, :]).T.copy()
        bff1q[c4] = ff1_w[sl, :] @ b2 + ff1_b[sl]
        ind4x[c4, sl] = 1.0
        d[f"wff2_{c4}"] = ff2_w[:, sl].T.copy()
    d["bff1q"] = bff1q
    d["ind4x"] = ind4x
    d["ones_col"] = np.ones((128, 1), np.float32)
    d["bff2_row"] = ff2_b[None, :].copy()

    for nm, wkey, bkey in (("wx1", "xcn_w1", "xcn_b1"),
                           ("wxj1", "xij_w1", "xij_b1")):
        W, B = f(wkey), f(bkey)
        for oc in range(2):
            sl = slice(128 * oc, 128 * (oc + 1))
            d[f"{nm}_{oc}"] = W[sl, :].T.copy()
            d[f"b{nm[1:]}_{oc}"] = B[sl][:, None].copy()
    for nm, wkey, bkey in (("wx2", "xcn_w2", "xcn_b2"),
                           ("wx3", "xcn_w3", "xcn_b3"),
                           ("wxj2", "xij_w2", "xij_b2"),
                           ("wl1", "lin_w1", "lin_b1")):
        W, B = f(wkey), f(bkey)
        for ic in range(2):
            for oc in range(2):
                d[f"{nm}_{ic}{oc}"] = \
                    W[128 * oc:128 * (oc + 1), 128 * ic:128 * (ic + 1)].T.copy()
        for oc in range(2):
            d[f"b{nm[1:]}_{oc}"] = B[128 * oc:128 * (oc + 1)][:, None].copy()
    lin_w2, lin_b2 = f("lin_w2"), f("lin_b2")
    d["wl2_0"] = lin_w2[0, :128][:, None].copy()
    d["wl2_1"] = lin_w2[0, 128:][:, None].copy()
    d["bl2"] = lin_b2.reshape(1, 1).copy()

    sel4 = np.zeros((4, 128), np.float32)
    for e in range(4):
        sel4[e, 32 * e:32 * (e + 1)] = 1.0
    d["sel4"] = sel4
    sel_all = np.zeros((128, 32 * 128), np.float32)
    for u in range(32):
        for t in range(128):
            sel_all[4 * u + t // 32, 128 * u + t] = 1.0
    d["sel_all"] = sel_all
    sel17 = np.zeros((1, 137), np.float32)
    for h in range(8):
        sel17[0, 17 * h + 16] = 1.0
    d["sel17"] = sel17
    d["ones1"] = np.ones((1, 128), np.float32)
    d["eps_col"] = np.full((128, 1), 1e-5, np.float32)
    d["epsd_col"] = np.full((128, 1), 1e-30, np.float32)
    d["beta_col"] = np.full((128, 1), np.asarray(inp["beta"],
                                                 np.float32).reshape(-1)[0])
    f32_keys = {"eps_col", "epsd_col", "beta_col", "bl2",
                "bx1_0", "bx1_1", "bx2_0", "bx2_1", "bx3_0", "bx3_1",
                "bxj1_0", "bxj1_1", "bxj2_0", "bxj2_1", "bl1_0", "bl1_1"}
    return {k: np.ascontiguousarray(
                np.asarray(v, np.float32) if k in f32_keys
                else np.asarray(v, np.float32).astype(BF_NP))
            for k, v in d.items()}


def _prep_core(inp, core, nt=NT):
    ne = 4 * nt // 128
    sl = slice(core * TC, (core + 1) * TC)
    tar = np.asarray(inp["tar_ei"])[:, sl].astype(np.int32)
    cols = np.asarray(inp["cn_cols"])[sl].astype(np.int32)     # [TC, K]
    cnt = np.asarray(inp["cn_counts"])[sl].astype(np.int64)    # [TC]

    d = {}
    d["idx_cn"] = np.ascontiguousarray(cols.reshape(-1).reshape(NT, 128).T)[:, :nt]
    d["idx_t0"] = np.ascontiguousarray(tar[0].reshape(NE, 128).T)[:, :ne]
    d["idx_t1"] = np.ascontiguousarray(tar[1].reshape(NE, 128).T)[:, :ne]

    kk = np.arange(K)
    valid_ek = (kk[None, :] < cnt[:, None])                    # [TC, K] bool
    valid_flat = valid_ek.reshape(-1)                          # token-order
    vf = np.ascontiguousarray(
        valid_flat.reshape(NT, 128).T.astype(np.float32))      # [128, NT]

    p = np.arange(128)
    pe = p // 32                                               # edge slot of row
    ebd = np.zeros((128, 4 * NT), np.float32)
    rc = (1.0 / np.maximum(cnt, 1)).astype(np.float32)         # [TC]
    for e in range(4):
        onblk = (pe == e)                                      # [128]
        ebd[:, e::4] = (onblk[:, None] * vf) * rc.reshape(NT, 4).T[e][None, :]
    d["vselfneg"] = (NEG * (1.0 - vf[:, :nt])).astype(np.float32)
    d["ebd"] = ebd[:, :4 * nt].astype(BF_NP)
    d["ind"] = (cnt > 0).astype(BF_NP)[None, :4 * nt].copy()
    return {k: np.ascontiguousarray(v) for k, v in d.items()}


_CACHE = {}
_CACHE_LOCK = threading.Lock()


def _get_nc(nt=NT, phases="abc", bsteps=99):
    with _CACHE_LOCK:
        key = (nt, phases, bsteps)
        if key not in _CACHE:
            _CACHE[key] = _build_nc(nt, phases, bsteps)
        return _CACHE[key]


def run(inputs, nt=NT, phases="abc", bsteps=99, **spmd_kwargs):
    """Run the kernel on the first 4*nt edges of each core's shard.

    Returns (out [NCORES, 4*nt], BassKernelResults).
    """
    nc = _get_nc(nt, phases, bsteps)
    shared = _prep_shared(inputs)
    x = np.ascontiguousarray(np.asarray(inputs["x"], np.float32))
    in_maps = []
    for core in range(NCORES):
        m = dict(shared)
        m["x"] = x
        m.update(_prep_core(inputs, core, nt))
        in_maps.append(m)
    res = run_bass_kernel_spmd(nc, in_maps, core_ids=list(range(NCORES)),
                               **spmd_kwargs)
    out = np.stack([res.results[c]["out"][0] for c in range(NCORES)])
    return out, res


def kernel(**inputs):
    out, _ = run(inputs)
    return out.reshape(T, O).astype(np.float32)

